# revision 1
# baseline (speedup 1.0000x reference)
"""GATv2 layer (100k nodes, 800k edges, 8 heads x 16 dim) on 8 Trainium2 cores.

Sharding: destination nodes are partitioned across the 8 cores (12.5k each).
Edges (with self-loops) are sorted by destination and assigned to the core
owning their destination.  Each core builds the full source-projection table
xl = x @ W_l + b_l (replicated compute - cheaper than an AllGather at the
~62 GB/s on-chip collective rate), then processes its edges in 512-edge
tiles: indirect-DMA row gathers of xl[src] / xr[dst], LeakyReLU + per-head
attention logits, exp (no max-subtraction: logits are O(0.3) so exp is safe
in fp32 and the softmax is mathematically identical), and a segment-softmax
reduction expressed as matmuls against 0/1 selection matrices so the
per-destination sums run on the tensor engine.  A fused epilogue
(normalize, +bias, ELU, +residual, LayerNorm) finishes owned rows, which
the host concatenates.

The single SPMD program is shared by all 8 cores; all per-core behavior
comes from per-core input tensors (x_own slice, edge index arrays).  The
static tile schedule is the per-group max across cores, with padding slots
marked dst_local=-1 so they match no selection-matrix column and contribute
nothing.
"""

import math

import numpy as np

P = 128
H, D = 8, 16
IN = 128
OUT = 128
NEG_SLOPE = 0.2
LN_EPS = 1e-5
DEN_EPS = 1e-16

N_CORES = 8


# ---------------------------------------------------------------------------
# CPU preprocessing: graph partitioning + static SPMD schedule
# ---------------------------------------------------------------------------

def _preprocess(edge_index: np.ndarray, n_nodes: int, n_cores: int):
    """Partition dst-sorted edges into a static per-core tile schedule.

    Returns (sched, idx_arrays, per):
      sched: per 128-edge subtile, (group, is_start, is_stop); shared by all
             cores (SPMD: one program).
      idx_arrays: per-core int32 [n_sub//4*128, 12]:
             cols 0-3  xl gather idx (global node id) for subtiles j=0..3
             cols 4-7  xr gather idx (core-local dst id)
             cols 8-11 dst-local-in-group as float32 bit pattern (-1 = pad)
      per: owned dst count per core.
    """
    src = edge_index[0].astype(np.int64)
    dst = edge_index[1].astype(np.int64)
    loops = np.arange(n_nodes, dtype=np.int64)
    src = np.concatenate([src, loops])
    dst = np.concatenate([dst, loops])
    order = np.argsort(dst, kind="stable")
    src = src[order]
    dst = dst[order]

    assert n_nodes % n_cores == 0
    per = n_nodes // n_cores
    n_groups = math.ceil(per / P)

    # group boundary edge offsets per core: lo/hi of (c, g)
    lo_b = np.empty((n_cores, n_groups), dtype=np.int64)
    hi_b = np.empty((n_cores, n_groups), dtype=np.int64)
    for c in range(n_cores):
        gb = np.minimum(c * per + np.arange(n_groups + 1) * P, (c + 1) * per)
        b = np.searchsorted(dst, gb)
        lo_b[c] = b[:-1]
        hi_b[c] = b[1:]
    cnt = hi_b - lo_b

    t_sub = np.maximum(1, np.ceil(cnt / P).astype(np.int64).max(axis=0))
    n_sub = int(t_sub.sum())
    t_sub[-1] += (-n_sub) % 4  # dummy subtiles pad the last group to 4 | n_sub
    n_sub = int(t_sub.sum())

    sched = []
    for g in range(n_groups):
        for k in range(int(t_sub[g])):
            sched.append((g, k == 0, k == int(t_sub[g]) - 1))

    idx_arrays = []
    for c in range(n_cores):
        arr = np.zeros((n_sub * P, 3), dtype=np.int64)
        arr[:, 2] = -1  # dst_local = -1 -> padding slot
        s = 0
        for g in range(n_groups):
            lo, hi = lo_b[c, g], hi_b[c, g]
            n_e = hi - lo
            arr[s : s + n_e, 0] = src[lo:hi]
            arr[s : s + n_e, 1] = dst[lo:hi] - c * per
            arr[s : s + n_e, 2] = dst[lo:hi] - (c * per + g * P)
            s += int(t_sub[g]) * P
        assert s == n_sub * P
        # pack to [n_tiles, 128, 12]: subtile j of tile t = slots t*512+j*128+p
        n_tiles = n_sub // 4
        a4 = arr.reshape(n_tiles, 4, P, 3)
        packed = np.zeros((n_tiles, P, 12), dtype=np.int32)
        packed[:, :, 0:4] = a4[:, :, :, 0].transpose(0, 2, 1)
        packed[:, :, 4:8] = a4[:, :, :, 1].transpose(0, 2, 1)
        dstloc = a4[:, :, :, 2].transpose(0, 2, 1).astype(np.float32)
        packed[:, :, 8:12] = dstloc.view(np.int32)
        idx_arrays.append(packed.reshape(n_tiles * P, 12))

    return sched, idx_arrays, per


# ---------------------------------------------------------------------------
# Bass program (shared by all cores; per-core behavior comes from inputs)
# ---------------------------------------------------------------------------

def _build_program(n_nodes, per, sched, use_blr, use_bias, use_gamma,
                   use_beta, lrelu_via_act=True, fused_gather=True):
    from contextlib import ExitStack

    from concourse import bass, mybir
    from concourse import tile as tile_mod
    from concourse.bacc import Bacc

    f32 = mybir.dt.float32
    i32 = mybir.dt.int32
    Alu = mybir.AluOpType
    Act = mybir.ActivationFunctionType
    TileContext = tile_mod.TileContext

    n_groups = math.ceil(per / P)
    own_pad = n_groups * P
    last_rows = per - (n_groups - 1) * P
    n_sub = len(sched)
    n_tiles2 = n_sub // 4
    tab_rows = n_nodes

    nc = Bacc()

    x_full = nc.declare_dram_parameter("x_full", [n_nodes, IN], f32, isOutput=False)
    x_own = nc.declare_dram_parameter("x_own", [own_pad, IN], f32, isOutput=False)
    w_l = nc.declare_dram_parameter("w_l", [IN, OUT], f32, isOutput=False)
    w_r = nc.declare_dram_parameter("w_r", [IN, OUT], f32, isOutput=False)
    idx_d = nc.declare_dram_parameter("idx", [n_tiles2 * P, 12], i32, isOutput=False)
    att_d = nc.declare_dram_parameter("att_b", [P, OUT], f32, isOutput=False)
    iota_d = nc.declare_dram_parameter("iota_b", [P, P], f32, isOutput=False)
    ident_d = nc.declare_dram_parameter("ident", [P, P], f32, isOutput=False)
    b4_d = nc.declare_dram_parameter("b4", [32, 544], f32, isOutput=False)
    aff_d = nc.declare_dram_parameter("aff", [P, 3 * OUT], f32, isOutput=False)
    blr_d = nc.declare_dram_parameter("blr", [2, OUT], f32, isOutput=False)
    out_own = nc.declare_dram_parameter("out_own", [own_pad, OUT], f32,
                                        isOutput=True)

    xl_tab = nc.dram_tensor("xl_tab", [tab_rows, OUT], f32)
    xr_tab = nc.dram_tensor("xr_tab", [own_pad, OUT], f32)

    with TileContext(nc) as tc, ExitStack() as ctx:
        const = ctx.enter_context(tc.tile_pool(name="const", bufs=1))
        wl_s = const.tile([IN, OUT], f32)
        wr_s = const.tile([IN, OUT], f32)
        att_s = const.tile([P, OUT], f32)
        iota_s = const.tile([P, P], f32)
        ident_s = const.tile([P, P], f32)
        b4_s = const.tile([32, 544], f32)
        aff_s = const.tile([P, 3 * OUT], f32)
        nc.sync.dma_start(out=wl_s[:], in_=w_l[:])
        nc.sync.dma_start(out=wr_s[:], in_=w_r[:])
        nc.sync.dma_start(out=att_s[:], in_=att_d[:])
        nc.sync.dma_start(out=iota_s[:], in_=iota_d[:])
        nc.sync.dma_start(out=ident_s[:], in_=ident_d[:])
        nc.sync.dma_start(out=b4_s[:], in_=b4_d[:])
        nc.sync.dma_start(out=aff_s[:], in_=aff_d[:])
        blr_s = None
        ones_s = None
        if use_blr:
            blr_s = const.tile([2, OUT], f32)
            ones_s = const.tile([1, P], f32)
            nc.sync.dma_start(out=blr_s[:], in_=blr_d[:])
            nc.vector.memset(ones_s[:], 1.0)

        # PE warmup: observe each DMA-loaded constant once on the PE clock,
        # so later matmuls never need two sync waits (PE LDWEIGHTS encodes
        # only one wait condition).
        with tc.tile_pool(name="warm", bufs=1, space="PSUM") as warm:
            warm_p = warm.tile([P, 512], f32)
            nc.tensor.transpose(out=warm_p[:, :P], in_=ident_s[:],
                                identity=ident_s[:])
            nc.tensor.matmul(out=warm_p[:, :OUT], lhsT=ident_s[:],
                             rhs=wl_s[:], start=True, stop=True)
            nc.tensor.matmul(out=warm_p[:, :OUT], lhsT=ident_s[:],
                             rhs=wr_s[:], start=True, stop=True)
            nc.tensor.matmul(out=warm_p[:32, :512], lhsT=ident_s[:32, :32],
                             rhs=b4_s[:, :512], start=True, stop=True)
            if use_blr:
                nc.tensor.matmul(out=warm_p[:2, :OUT], lhsT=ident_s[:2, :2],
                                 rhs=blr_s[:], start=True, stop=True)

        # ------------------------------------------------------------------
        # Phase 1: projection tables (xT tile -> matmul -> table row block)
        # ------------------------------------------------------------------
        def project(src_ap, dst_ap, w_tile, n_rows, bias_row):
            with tc.tile_pool(name="p1", bufs=3) as p1, \
                 tc.tile_pool(name="p1ps", bufs=2, space="PSUM") as p1ps:
                n_blk = (n_rows + 511) // 512
                for b in range(n_blk):
                    r0 = b * 512
                    rows = min(512, n_rows - r0)
                    n_j = (rows + P - 1) // P
                    xin = p1.tile([P, 4, IN], f32, tag="xin")
                    stg = p1.tile([P, 4, OUT], f32, tag="stg")
                    # one block DMA per 512 rows (Bacc's event-semaphore pass
                    # legalizes the resulting multi-wait transposes)
                    if rows == 512:
                        nc.sync.dma_start(
                            out=xin[:],
                            in_=src_ap[r0 : r0 + 512, :].rearrange(
                                "(j p) f -> p j f", p=P
                            ),
                        )
                    else:
                        for j in range(n_j):
                            jr = min(P, rows - j * P)
                            nc.sync.dma_start(
                                out=xin[:jr, j, :],
                                in_=src_ap[r0 + j * P : r0 + j * P + jr, :],
                            )
                    for j in range(n_j):
                        jr = min(P, rows - j * P)
                        xt_p = p1ps.tile([P, P], f32, tag="xt")
                        nc.tensor.transpose(
                            out=xt_p[:, :jr],
                            in_=xin[:jr, j, :],
                            identity=ident_s[:jr, :jr],
                        )
                        xt_s = p1.tile([P, P], f32, tag="xts")
                        nc.scalar.copy(out=xt_s[:, :jr], in_=xt_p[:, :jr])
                        o_p = p1ps.tile([P, OUT], f32, tag="op")
                        nc.tensor.matmul(
                            out=o_p[:jr, :],
                            lhsT=xt_s[:, :jr],
                            rhs=w_tile[:],
                            start=True,
                            stop=bias_row is None,
                        )
                        if bias_row is not None:
                            nc.tensor.matmul(
                                out=o_p[:jr, :],
                                lhsT=ones_s[:1, :jr],
                                rhs=bias_row,
                                start=False,
                                stop=True,
                            )
                        nc.vector.tensor_copy(out=stg[:jr, j, :], in_=o_p[:jr, :])
                    if rows == 512:
                        nc.sync.dma_start(
                            out=dst_ap[r0 : r0 + 512, :].rearrange(
                                "(j p) f -> p j f", p=P
                            ),
                            in_=stg[:],
                        )
                    else:
                        for j in range(n_j):
                            jr = min(P, rows - j * P)
                            nc.sync.dma_start(
                                out=dst_ap[r0 + j * P : r0 + j * P + jr, :],
                                in_=stg[:jr, j, :],
                            )

        project(x_full[:], xl_tab[: n_nodes, :], wl_s, n_nodes,
                blr_s[0:1, :] if use_blr else None)
        project(x_own[:], xr_tab[:], wr_s, own_pad,
                blr_s[1:2, :] if use_blr else None)

        # ------------------------------------------------------------------
        # Phase 2: edge tiles + segment softmax + fused epilogue
        # ------------------------------------------------------------------
        with tc.tile_pool(name="p2", bufs=3) as p2, \
             tc.tile_pool(name="p2w", bufs=3) as p2w, \
             tc.tile_pool(name="gps", bufs=2, space="PSUM") as gps, \
             tc.tile_pool(name="mps", bufs=2, space="PSUM") as mps, \
             tc.tile_pool(name="eps", bufs=1, space="PSUM") as epsp, \
             tc.tile_pool(name="stp", bufs=2) as stp:

            att_b4 = att_s[:][:, None, :].to_broadcast((P, 4, OUT))
            iota_b4 = iota_s[:][:, None, :].to_broadcast((P, 4, P))

            state = {"stage": None, "groups": []}
            g_psum = {}

            def run_epilogue():
                stage = state["stage"]
                groups = state["groups"]
                nw = len(groups)
                num = stage[:, :nw, 0:OUT]
                den = stage[:, :nw, OUT : OUT + H]
                rd = p2.tile([P, 4, H], f32, tag="rd")
                nc.vector.tensor_scalar_add(rd[:, :nw, :], den, DEN_EPS)
                nc.vector.reciprocal(rd[:, :nw, :], rd[:, :nw, :])
                o1 = p2.tile([P, 4, OUT], f32, tag="o1")
                nc.vector.tensor_tensor(
                    out=o1[:, :nw, :].rearrange("p j (h d) -> p j h d", h=H),
                    in0=num.rearrange("p j (h d) -> p j h d", h=H),
                    in1=rd[:, :nw, :, None].to_broadcast((P, nw, H, D)),
                    op=Alu.mult,
                )
                if use_bias:
                    nc.vector.tensor_tensor(
                        out=o1[:, :nw, :],
                        in0=o1[:, :nw, :],
                        in1=aff_s[:][:, None, 2 * OUT : 3 * OUT].to_broadcast(
                            (P, nw, OUT)
                        ),
                        op=Alu.add,
                    )
                # ELU(v) = max(v,0)-1 + exp(min(v,0)); then + residual x
                vmin = p2.tile([P, 4, OUT], f32, tag="vmin")
                nc.vector.tensor_scalar_min(vmin[:, :nw, :], o1[:, :nw, :], 0.0)
                ev = p2.tile([P, 4, OUT], f32, tag="ev")
                nc.scalar.activation(out=ev[:, :nw, :], in_=vmin[:, :nw, :],
                                     func=Act.Exp)
                vm1 = p2.tile([P, 4, OUT], f32, tag="vm1")
                nc.vector.tensor_scalar(
                    out=vm1[:, :nw, :], in0=o1[:, :nw, :],
                    scalar1=0.0, scalar2=-1.0, op0=Alu.max, op1=Alu.add,
                )
                xres = p2.tile([P, 4, OUT], f32, tag="xres")
                for j, g in enumerate(groups):
                    nc.sync.dma_start(
                        out=xres[:, j, :], in_=x_own[g * P : (g + 1) * P, :]
                    )
                v = p2.tile([P, 4, OUT], f32, tag="v")
                nc.vector.tensor_tensor(out=v[:, :nw, :], in0=ev[:, :nw, :],
                                        in1=vm1[:, :nw, :], op=Alu.add)
                nc.vector.tensor_tensor(out=v[:, :nw, :], in0=v[:, :nw, :],
                                        in1=xres[:, :nw, :], op=Alu.add)
                # LayerNorm over the feature dim
                mu = p2.tile([P, 4], f32, tag="mu")
                nc.vector.tensor_reduce(out=mu[:, :nw], in_=v[:, :nw, :],
                                        axis=mybir.AxisListType.X, op=Alu.add)
                nc.vector.tensor_scalar_mul(mu[:, :nw], mu[:, :nw], 1.0 / OUT)
                cen = p2.tile([P, 4, OUT], f32, tag="cen")
                nc.vector.tensor_tensor(
                    out=cen[:, :nw, :], in0=v[:, :nw, :],
                    in1=mu[:, :nw, None].to_broadcast((P, nw, OUT)),
                    op=Alu.subtract,
                )
                sq = p2.tile([P, 4, OUT], f32, tag="sq")
                nc.vector.tensor_tensor(out=sq[:, :nw, :], in0=cen[:, :nw, :],
                                        in1=cen[:, :nw, :], op=Alu.mult)
                var = p2.tile([P, 4], f32, tag="var")
                nc.vector.tensor_reduce(out=var[:, :nw], in_=sq[:, :nw, :],
                                        axis=mybir.AxisListType.X, op=Alu.add)
                nc.vector.tensor_scalar(
                    out=var[:, :nw], in0=var[:, :nw],
                    scalar1=1.0 / OUT, scalar2=LN_EPS,
                    op0=Alu.mult, op1=Alu.add,
                )
                nc.scalar.activation(out=var[:, :nw], in_=var[:, :nw],
                                     func=Act.Sqrt)
                nc.vector.reciprocal(var[:, :nw], var[:, :nw])
                o2 = p2.tile([P, 4, OUT], f32, tag="o2")
                nc.vector.tensor_tensor(
                    out=o2[:, :nw, :], in0=cen[:, :nw, :],
                    in1=var[:, :nw, None].to_broadcast((P, nw, OUT)),
                    op=Alu.mult,
                )
                if use_gamma:
                    nc.vector.tensor_tensor(
                        out=o2[:, :nw, :], in0=o2[:, :nw, :],
                        in1=aff_s[:][:, None, 0:OUT].to_broadcast((P, nw, OUT)),
                        op=Alu.mult,
                    )
                if use_beta:
                    nc.vector.tensor_tensor(
                        out=o2[:, :nw, :], in0=o2[:, :nw, :],
                        in1=aff_s[:][:, None, OUT : 2 * OUT].to_broadcast(
                            (P, nw, OUT)
                        ),
                        op=Alu.add,
                    )
                for j, g in enumerate(groups):
                    gr = last_rows if g == n_groups - 1 else P
                    nc.sync.dma_start(
                        out=out_own[g * P : g * P + gr, :], in_=o2[:gr, j, :]
                    )
                state["stage"] = None
                state["groups"] = []

            for t in range(n_tiles2):
                subs = sched[t * 4 : t * 4 + 4]
                idx_t = p2.tile([P, 12], i32, tag="idx")
                nc.sync.dma_start(out=idx_t[:],
                                  in_=idx_d[t * P : (t + 1) * P, :])
                xl_e = p2w.tile([P, 4, 136], f32, tag="xle")
                xr_e = p2w.tile([P, 4, OUT], f32, tag="xre")
                if fused_gather:
                    nc.gpsimd.indirect_dma_start(
                        out=xl_e[:, :, 0:OUT],
                        out_offset=None,
                        in_=xl_tab[:],
                        in_offset=bass.IndirectOffsetOnAxis(
                            ap=idx_t[:, 0:4], axis=0
                        ),
                    )
                    nc.gpsimd.indirect_dma_start(
                        out=xr_e[:],
                        out_offset=None,
                        in_=xr_tab[:],
                        in_offset=bass.IndirectOffsetOnAxis(
                            ap=idx_t[:, 4:8], axis=0
                        ),
                    )
                else:
                    for j in range(4):
                        nc.gpsimd.indirect_dma_start(
                            out=xl_e[:, j, 0:OUT],
                            out_offset=None,
                            in_=xl_tab[:],
                            in_offset=bass.IndirectOffsetOnAxis(
                                ap=idx_t[:, j : j + 1], axis=0
                            ),
                        )
                        nc.gpsimd.indirect_dma_start(
                            out=xr_e[:, j, :],
                            out_offset=None,
                            in_=xr_tab[:],
                            in_offset=bass.IndirectOffsetOnAxis(
                                ap=idx_t[:, 4 + j : 5 + j], axis=0
                            ),
                        )
                nc.gpsimd.memset(xl_e[:, :, OUT:136], 1.0)
                # m = xl + xr on the tensor engine: two N=512 identity
                # matmuls accumulating into one PSUM bank (vs 8 N=128 ones -
                # PE slice count was the measured bottleneck)
                m_p = mps.tile([P, 4, OUT], f32, tag="mp")
                nc.tensor.matmul(out=m_p[:], lhsT=ident_s[:],
                                 rhs=xl_e[:, :, 0:OUT], start=True,
                                 stop=False)
                nc.tensor.matmul(out=m_p[:], lhsT=ident_s[:],
                                 rhs=xr_e[:], start=False, stop=True)
                # lrelu(x) = 0.2x + relu(0.8x); ACT Lrelu's alpha immediate
                # is broken on HW, so split across two ACT ops + one DVE add
                t_s = p2w.tile([P, 4, OUT], f32, tag="ts")
                if lrelu_via_act:
                    r1 = p2w.tile([P, 4, OUT], f32, tag="r1")
                    r2 = p2w.tile([P, 4, OUT], f32, tag="r2")
                    nc.scalar.activation(out=r1[:], in_=m_p[:], func=Act.Relu,
                                         scale=1.0 - NEG_SLOPE)
                    nc.scalar.mul(out=r2[:], in_=m_p[:], mul=NEG_SLOPE)
                    nc.vector.tensor_tensor(out=t_s[:], in0=r1[:], in1=r2[:],
                                            op=Alu.add)
                else:
                    # exact fallback: lrelu(x) = max(x, alpha*x)
                    tsc = p2w.tile([P, 4, OUT], f32, tag="tsc")
                    nc.vector.tensor_scalar_mul(tsc[:], m_p[:], NEG_SLOPE)
                    nc.vector.tensor_tensor(out=t_s[:], in0=m_p[:],
                                            in1=tsc[:], op=Alu.max)
                u = p2w.tile([P, 4, OUT], f32, tag="u")
                nc.vector.tensor_tensor(out=u[:], in0=t_s[:], in1=att_b4,
                                        op=Alu.mult)
                e = p2.tile([P, 4, H], f32, tag="e")
                nc.vector.tensor_reduce(
                    out=e[:].rearrange("p j h -> p (j h)"),
                    in_=u[:].rearrange("p j (h d) -> p (j h) d", h=H),
                    axis=mybir.AxisListType.X,
                    op=Alu.add,
                )
                ex = p2.tile([P, 4, H], f32, tag="ex")
                nc.scalar.activation(out=ex[:], in_=e[:], func=Act.Exp)
                ext_p = epsp.tile([32, P], f32, tag="extp")
                nc.tensor.transpose(
                    out=ext_p[:],
                    in_=ex[:].rearrange("p j h -> p (j h)"),
                    identity=ident_s[:],
                )
                ext_s = p2.tile([32, P], f32, tag="exts")
                nc.vector.tensor_copy(out=ext_s[:], in_=ext_p[:])
                ew_p = epsp.tile([P, 544], f32, tag="ewp")
                nc.tensor.matmul(out=ew_p[:, 0:512], lhsT=ext_s[:],
                                 rhs=b4_s[:, 0:512], start=True, stop=True)
                nc.tensor.matmul(out=ew_p[:, 512:544], lhsT=ext_s[:],
                                 rhs=b4_s[:, 512:544], start=True, stop=True)
                w_t = p2w.tile([P, 4, 136], f32, tag="wt")
                nc.vector.tensor_tensor(
                    out=w_t[:],
                    in0=ew_p[:].rearrange("p (j c) -> p j c", j=4),
                    in1=xl_e[:],
                    op=Alu.mult,
                )
                s4 = p2w.tile([P, 4, P], f32, tag="s4")
                nc.vector.tensor_tensor(
                    out=s4[:],
                    in0=iota_b4,
                    in1=idx_t[:, 8:12].bitcast(f32)[:, :, None].to_broadcast(
                        (P, 4, P)
                    ),
                    op=Alu.is_equal,
                )
                for j, (g, is_start, is_stop) in enumerate(subs):
                    if is_start:
                        g_psum[g] = gps.tile([P, 136], f32, tag="gp", name=f"gp{g}")
                    nc.tensor.matmul(
                        out=g_psum[g][:],
                        lhsT=s4[:, j, :],
                        rhs=w_t[:, j, :],
                        start=is_start,
                        stop=is_stop,
                    )
                    if is_stop:
                        if state["stage"] is None:
                            state["stage"] = stp.tile(
                                [P, 4, 136], f32, tag="stage", name=f"stage{t}"
                            )
                        slot = len(state["groups"])
                        nc.scalar.copy(out=state["stage"][:, slot, :],
                                       in_=g_psum[g][:])
                        state["groups"].append(g)
                        del g_psum[g]
                        if len(state["groups"]) == 4 or g == n_groups - 1:
                            run_epilogue()
            assert state["stage"] is None and not g_psum

    nc.finalize()
    return nc


# ---------------------------------------------------------------------------
# Host entry point
# ---------------------------------------------------------------------------

def _make_consts(att, bias, gamma, beta, b_l, b_r):
    att_b = np.tile(np.asarray(att, np.float32).reshape(1, OUT), (P, 1))
    iota_b = np.tile(np.arange(P, dtype=np.float32)[None, :], (P, 1))
    ident = np.eye(P, dtype=np.float32)
    b4 = np.zeros((32, 544), dtype=np.float32)
    for j in range(4):
        for h in range(H):
            row = j * H + h
            b4[row, j * 136 + h * D : j * 136 + (h + 1) * D] = 1.0
            b4[row, j * 136 + OUT + h] = 1.0
    aff = np.zeros((P, 3 * OUT), dtype=np.float32)
    aff[:, 0:OUT] = np.asarray(gamma, np.float32)[None, :]
    aff[:, OUT : 2 * OUT] = np.asarray(beta, np.float32)[None, :]
    aff[:, 2 * OUT : 3 * OUT] = np.asarray(bias, np.float32)[None, :]
    blr = np.stack([np.asarray(b_l, np.float32), np.asarray(b_r, np.float32)])
    return att_b, iota_b, ident, b4, aff, blr


TRACE = False       # set by test harness to collect an NTFF profile
LAST = {}           # stash of the last BassKernelResults (for test.py)
# HW ACT Lrelu does not honor the alpha immediate (measured 1.3% error);
# use the exact 2-op DVE fallback until a working ACT formulation is found.
LRELU_VIA_ACT = True
FUSED_GATHER = False  # multi-offset indirect DMA crashes the exec unit on HW


def kernel(x, edge_index, W_l, b_l, W_r, b_r, att, bias, gamma, beta):
    from concourse.bass_utils import run_bass_kernel_spmd

    x = np.asarray(x, dtype=np.float32)
    edge_index = np.asarray(edge_index)
    n_nodes = x.shape[0]
    n_cores = N_CORES

    sched, idx_arrays, per = _preprocess(edge_index, n_nodes, n_cores)
    n_groups = math.ceil(per / P)
    own_pad = n_groups * P

    use_blr = bool(np.any(b_l)) or bool(np.any(b_r))
    use_bias = bool(np.any(bias))
    use_gamma = bool(np.any(np.asarray(gamma) != 1.0))
    use_beta = bool(np.any(beta))

    nc = _build_program(n_nodes, per, sched, use_blr, use_bias, use_gamma,
                        use_beta, lrelu_via_act=LRELU_VIA_ACT,
                        fused_gather=FUSED_GATHER)

    att_b, iota_b, ident, b4, aff, blr = _make_consts(att, bias, gamma, beta,
                                                      b_l, b_r)

    in_maps = []
    for c in range(n_cores):
        x_own = np.zeros((own_pad, IN), dtype=np.float32)
        x_own[:per] = x[c * per : (c + 1) * per]
        in_maps.append({
            "x_full": x,
            "x_own": x_own,
            "w_l": np.asarray(W_l, dtype=np.float32),
            "w_r": np.asarray(W_r, dtype=np.float32),
            "idx": idx_arrays[c],
            "att_b": att_b,
            "iota_b": iota_b,
            "ident": ident,
            "b4": b4,
            "aff": aff,
            "blr": blr,
        })

    res = run_bass_kernel_spmd(nc, in_maps, list(range(n_cores)), trace=TRACE)
    LAST["res"] = res
    outs = [res.results[c]["out_own"][:per] for c in range(n_cores)]
    return np.concatenate(outs, axis=0).astype(np.float32)

